# revision 48
# baseline (speedup 1.0000x reference)
"""NePuEncoder Bass/Tile kernel for 8 Trainium2 NeuronCores.

Sharding: query-parallel. Core c handles batch b=c//4, queries qo=(c%4)*96 ..
qo+96 of that batch. Channel-major layout [128 chan, keys] throughout.

Per-channel softmax attention fully fused in SBUF/PSUM:
  - pairwise trig features via range-reduced Sin (|arg|<=pi): 32 chunks of 3
    queries, computed once and reused across all 3 transformer blocks. The
    r = s*(xk-xq)+off matmul runs in bf16 via split-precision (z_hi/z_lo +
    c_hi/c_lo rhs rows against 0/1-selector lhsT columns, K=32) instead of
    fp32 (4 cyc/row). Range reduction is split across engines: round-to-int
    on ACT (Identity + C_ROUND bias, f32 store), -frac via one DVE
    scalar_tensor_tensor, Sin with scale=-2pi; two chunks per instruction
    over 2-bank PSUM tiles. First-24-query relus of block 0 pre-run on DVE
    in this phase's shadow.
  - main loop in query PAIRS with a software-pipelined two-stage lag: at
    iteration p the relus+logit matmuls of pair p are emitted, the Exp of
    pair p-1 (one [128,2,384] strided activation over both PSUM banks, no
    accum_out), and the s0/stt drain of pair p-2 -- so neither ACT nor DVE
    ever head-of-line blocks on an in-flight cross-engine producer. The lag
    collapses at the final pair so the flush is just the last drains.
  - per query, S0 comes from a DVE tensor_scalar in 4x_2p mode (all-SBUF
    bf16 w -> scratch, accum_out=S0, ~160ns) instead of the 505+187ns
    unpaired exp+accumulator; S1 via DVE scalar_tensor_tensor (pos+QP)*w.
    Relu (per-query bias blocks pairing) alternates ACT/DVE: even queries
    on ACT, odd queries 9/16 on ACT, balancing all three engines at
    ~1.65us/pair.
  - PSUM: 8 banks as lg-pairs 2x[128,1024] + hpre ring 2x[128,512] + pos
    ring 2x[128,512]; trig pairs, MLP pairs and the final FC reuse the lg
    pool; make_consts reuses the hpre/pos rings.
  - two 8-rank AllGathers per block (query-halves): queries 0-83 are
    reduced, gathered, written back AND per-core bn_stats'd while 84-95
    still compute, so only the 12-query gather + stats sit on the block
    tail; BatchNorm rsqrt is a DVE-only shift-seed Newton iteration, and a
    single LoadActFuncSet pin keeps one table resident after the Sin phase.
    MLP blocks consume the gathered bf16 o_full directly: the BN scale is
    folded into em1 per input channel (63ns DVE op) and the BN shift into
    the layer-1 bias, while the residual is o2 = sc*o + y2 -- the dropped
    per-channel shift cancels exactly through the next BatchNorm's mean
    subtraction, so the 825ns f_full eviction leaves the tail critical
    path. Payload/gather/o_full are bf16 (half the collective bytes). The
    final FC (pair-wide psum, one 2-group max reduce) and MLPs are
    computed redundantly per core; core 0's output is returned.
  - warm_pe: dep-gated back-to-back dummy 384-col matmuls keep the tensor
    engine's p-state ramp alive through tail bubbles so consts / FC /
    next-block matmuls run at 1 cyc/row instead of the 2-3.7x post-idle
    penalty.

All constant tensors are packed host-side into 5 DRAM tensors loaded with 5
DMAs (split across SP/ACT/Pool queues) at program start.
"""
import sys

sys.path.insert(0, "/opt/trn_rl_repo")

import numpy as np
import ml_dtypes

B, N, D, DS, LAT, FD, NF = 2, 384, 128, 3, 256, 1, 2
NB = NF + 1
NQ = 96                  # queries per core
NCH = 32                 # trig chunks (3 queries each)
NPAIR = NQ // 2
FREQS = np.linspace(1.0, 32.0, 5).astype(np.float64)
EPS = 1e-5
TWO_PI = float(2 * np.pi)
C_ROUND = float(3 << 22)  # 2^23 + 2^22: fp32 round-to-nearest-even trick
PREQ = 24                # block-0 queries whose relu pre-runs in trig shadow

BF = ml_dtypes.bfloat16

# --- wpack column offsets (bf16 [128, WCOLS]) ---
W_WG1, W_WG2, W_NWK, W_WV, W_G1Q = 0, 384, 768, 1152, 1536
W_G1P, W_PS, W_I128 = 1920, 2304, 2688
W_EM1, W_EM2, W_F1, W_F2 = 2816, 3072, 3328, 3584
WCOLS = 4096
# --- vpack column offsets (f32 [128, VCOLS]) ---
V_BG2, V_GAM, V_BET, V_ENCB = 0, 3, 6, 9
V_EMB1, V_EMB2, V_EMG, V_EMBE, V_F1B, V_F2B = 10, 12, 14, 16, 18, 20
V_MAGIC = 22
VCOLS = 23
# --- rpack column offsets (bf16 [3, RCOLS]) ---
R_XKB, R_NPD4, R_G1PD4, R_PD4, R_XQB = 0, 384, 768, 1152, 1536
RCOLS = 1632
# --- qpack column offsets (bf16 [1, QCOLS]) ---
Q_FEATS, Q_ENC, Q_C1, Q_BPE, Q_FB, Q_FQ = 0, 768, 896, 1280, 1664, 2048
QCOLS = 2144
# --- fpack column offsets (bf16 [32, FCOLS]) ---
F_S4, F_XKO = 0, NCH * 96
FCOLS = NCH * 96 + 384


def _bf(x):
    return np.ascontiguousarray(np.asarray(x, np.float32).astype(BF))


def _f32(x):
    return np.ascontiguousarray(np.asarray(x, np.float32))


def _wpe_split(Wpe):
    """W_s [128,30] trig cols (sin-sign absorbed for npd = xk - xq), W_d [128,3]."""
    Ws = np.zeros((D, 30), np.float32)
    for j in range(3):
        for t in range(10):
            r = 10 * j + t
            if t < 5:
                Ws[:, r] = -Wpe[:, 3 + 6 * t + j]
            else:
                Ws[:, r] = Wpe[:, 3 + 6 * (t - 5) + 3 + j]
    return Ws, Wpe[:, 0:3].astype(np.float32)


def _rep3(M30):
    """[30,128] -> [128,128] with copies at partition offsets 0/32/64."""
    out = np.zeros((128, 128), np.float32)
    for s in range(3):
        out[32 * s:32 * s + 30, :] = M30
    return out


_CACHE = {}


def _build(variant="spmd"):
    if variant in _CACHE:
        return _CACHE[variant]

    import concourse.bacc as bacc
    import concourse.bass as bass
    import concourse.tile as tile
    from concourse import mybir

    F32, BF16, U32 = mybir.dt.float32, mybir.dt.bfloat16, mybir.dt.uint32
    AF = mybir.ActivationFunctionType
    OP = mybir.AluOpType

    nc = bacc.Bacc(None, target_bir_lowering=False,
                   num_devices=(8 if variant == "spmd" else 1))

    wpack_d = nc.dram_tensor("wpack", [128, WCOLS], BF16, kind="ExternalInput")
    vpack_d = nc.dram_tensor("vpack", [128, VCOLS], F32, kind="ExternalInput")
    rpack_d = nc.dram_tensor("rpack", [3, RCOLS], BF16, kind="ExternalInput")
    qpack_d = nc.dram_tensor("qpack", [1, QCOLS], BF16, kind="ExternalInput")
    fpack_d = nc.dram_tensor("fpack", [32, FCOLS], BF16, kind="ExternalInput")

    out_d = nc.dram_tensor("out", [2, 256], F32, kind="ExternalOutput")
    RG = [[0, 1, 2, 3, 4, 5, 6, 7]]

    def pair_view(t_ap, half_stride, width=384):
        """[128, 2, width] strided view of an AP (two halves at half_stride)."""
        return bass.AP(tensor=t_ap.tensor, offset=t_ap.offset,
                       ap=[list(t_ap.ap[0]), [half_stride, 2], [1, width]])

    with tile.TileContext(nc) as tc:
        with (
            tc.tile_pool(name="sing", bufs=1) as sing,
            tc.tile_pool(name="fpool", bufs=2) as fpool,
            tc.tile_pool(name="blk", bufs=2) as blk,
            tc.tile_pool(name="hp", bufs=30) as hp,
            tc.tile_pool(name="wp", bufs=3) as wp,
            tc.tile_pool(name="wide", bufs=2) as wide,
            tc.tile_pool(name="smalls", bufs=4) as smalls,
            tc.tile_pool(name="st1", bufs=3) as st1,
            tc.tile_pool(name="ps_l", bufs=2, space="PSUM") as ps_l,
            tc.tile_pool(name="ps_h", bufs=2, space="PSUM") as ps_h,
            tc.tile_pool(name="ps_p", bufs=2, space="PSUM") as ps_p,
            tc.tile_pool(name="dram", bufs=1, space="DRAM") as dram,
        ):
            # ---------- packed constant loads: 5 DMAs, issued first ----------
            fpack = sing.tile([32, FCOLS], BF16, tag="fpack", name="fpack")
            nc.sync.dma_start(out=fpack, in_=fpack_d[:])
            wpack = sing.tile([128, WCOLS], BF16, tag="wpack", name="wpack")
            nc.gpsimd.dma_start(out=wpack, in_=wpack_d[:])
            vpack = sing.tile([128, VCOLS], F32, tag="vpack", name="vpack")
            nc.scalar.dma_start(out=vpack, in_=vpack_d[:])
            rpack = sing.tile([3, RCOLS], BF16, tag="rpack", name="rpack")
            nc.scalar.dma_start(out=rpack, in_=rpack_d[:])
            qpack = sing.tile([1, QCOLS], BF16, tag="qpack", name="qpack")
            nc.sync.dma_start(out=qpack, in_=qpack_d[:])

            def wsl(off, i=0):
                return wpack[:, off + 128 * i: off + 128 * (i + 1)]

            W = {
                "Wg1": [wsl(W_WG1, i) for i in range(NB)],
                "Wg2": [wsl(W_WG2, i) for i in range(NB)],
                "nWk": [wsl(W_NWK, i) for i in range(NB)],
                "Wv": [wsl(W_WV, i) for i in range(NB)],
                "G1Q": [wsl(W_G1Q, i) for i in range(NB)],
                "i128": wsl(W_I128),
                "em1": [wsl(W_EM1, j) for j in range(NF)],
                "em2": [wsl(W_EM2, j) for j in range(NF)],
                "f1": [wsl(W_F1, h) for h in range(2)],
                "f2": [[wsl(W_F2, 2 * h + k) for k in range(2)] for h in range(2)],
                "bg2": [vpack[:, V_BG2 + i: V_BG2 + i + 1] for i in range(NB)],
                "gam": [vpack[:, V_GAM + i: V_GAM + i + 1] for i in range(NB)],
                "bet": [vpack[:, V_BET + i: V_BET + i + 1] for i in range(NB)],
                "encb": vpack[:, V_ENCB: V_ENCB + 1],
                "emb1": [vpack[:, V_EMB1 + j: V_EMB1 + j + 1] for j in range(NF)],
                "emb2": [vpack[:, V_EMB2 + j: V_EMB2 + j + 1] for j in range(NF)],
                "emg": [vpack[:, V_EMG + j: V_EMG + j + 1] for j in range(NF)],
                "embe": [vpack[:, V_EMBE + j: V_EMBE + j + 1] for j in range(NF)],
                "f1b": [vpack[:, V_F1B + h: V_F1B + h + 1] for h in range(2)],
                "f2b": [vpack[:, V_F2B + h: V_F2B + h + 1] for h in range(2)],
                "xkb": rpack[:, R_XKB: R_XKB + 384],
                "xqb": rpack[:, R_XQB: R_XQB + 96],
                "nPd4": [rpack[:, R_NPD4 + 128 * i: R_NPD4 + 128 * (i + 1)] for i in range(NB)],
                "G1Pd4": [rpack[:, R_G1PD4 + 128 * i: R_G1PD4 + 128 * (i + 1)] for i in range(NB)],
                "Pd4": [rpack[:, R_PD4 + 128 * i: R_PD4 + 128 * (i + 1)] for i in range(NB)],
                "feats": qpack[:, Q_FEATS: Q_FEATS + 768],
                "enc": qpack[:, Q_ENC: Q_ENC + 128],
                "c1": [qpack[:, Q_C1 + 128 * i: Q_C1 + 128 * (i + 1)] for i in range(NB)],
                "bpe": [qpack[:, Q_BPE + 128 * i: Q_BPE + 128 * (i + 1)] for i in range(NB)],
                "featsb": qpack[:, Q_FB: Q_FB + 384],
                "featsq": qpack[:, Q_FQ: Q_FQ + 96],
            }

            ones96 = sing.tile([1, 96], BF16, tag="ones96")
            nc.vector.memset(ones96, 1.0)
            crt = sing.tile([128, 1], F32, tag="crt")
            nc.vector.memset(crt, C_ROUND)
            ncrt = sing.tile([128, 1], F32, tag="ncrt")
            nc.vector.memset(ncrt, -C_ROUND)
            dumA = sing.tile([128, 1], BF16, tag="dumA")
            sdum = sing.tile([128, 384], BF16, tag="sdum")

            # 128 partitions so the tile is forced to base partition 0 (PE
            # lhsT/rhs base must be 0/32/64); rows 96-127 unused.
            trigc = sing.tile([128, NCH * 384], BF16, tag="trig", name="trig")

            def tsl_of(m):
                s4o = 32 * (m % 3)
                c = m // 3
                return trigc[s4o:s4o + 30, 384 * c: 384 * c + 384]

            # ---------- stage 1: trig features, bf16 split-precision ----------
            # r[row,k] = z_hi + z_lo (+ c_hi + c_lo) summed via one bf16
            # matmul (K=32); n = (r+C)-C (round); -frac = (n-C)-r handled as
            # nf = r - n; trig = sin(2pi*nf) -> bf16. Two chunks (8 queries)
            # per range-reduction / Sin instruction.
            xko = fpack[:, F_XKO: F_XKO + 384]

            def emit_trig_pair(ci):
                c0, c1 = 2 * ci, 2 * ci + 1
                rp = ps_l.tile([128, 1024], F32, tag="lg")
                nc.tensor.matmul(rp[0:96, 0:384],
                                 fpack[:, F_S4 + 96 * c0: F_S4 + 96 * c0 + 96],
                                 xko, start=True, stop=True)
                nc.tensor.matmul(rp[0:96, 512:896],
                                 fpack[:, F_S4 + 96 * c1: F_S4 + 96 * c1 + 96],
                                 xko, start=True, stop=True)
                rp96 = rp[0:96, :]
                rp_v = pair_view(rp96, 512)
                rpc = st1.tile([96, 768], F32, tag="nt")
                nc.scalar.activation(out=pair_view(rpc[:], 384), in_=rp_v,
                                     func=AF.Identity, bias=crt[0:96, 0:1],
                                     scale=1.0)
                nf = st1.tile([96, 768], F32, tag="nf")
                nc.vector.scalar_tensor_tensor(
                    out=pair_view(nf[:], 384), in0=pair_view(rpc[:], 384),
                    scalar=ncrt[0:96, 0:1], in1=rp_v, op0=OP.add,
                    op1=OP.subtract)
                nc.scalar.activation(out=trigc[0:96, 384 * c0: 384 * c0 + 768],
                                     in_=nf[:, 0:768], func=AF.Sin,
                                     bias=0.0, scale=-TWO_PI)

            # ---------- initial features ----------
            fb = fpool.tile([128, 384], BF16, tag="fb")
            p = ps_h.tile([128, 512], F32, tag="ph")
            nc.tensor.matmul(p[:, 0:384], W['enc'], W['featsb'], start=True, stop=True)
            nc.scalar.activation(out=fb, in_=p[:, 0:384], func=AF.Identity,
                                 bias=W['encb'], scale=1.0)
            fq = fpool.tile([128, 96], BF16, tag="fq")
            p = ps_h.tile([128, 512], F32, tag="ph")
            nc.tensor.matmul(p[:, 0:96], W['enc'], W['featsq'], start=True, stop=True)
            nc.scalar.activation(out=fq, in_=p[:, 0:96], func=AF.Identity,
                                 bias=W['encb'], scale=1.0)

            pid = nc.scalar.partition_id()

            def make_consts(i):
                pa = ps_h.tile([128, 512], F32, tag="ph")
                nc.tensor.matmul(pa[:, 0:384], W['nWk'][i], fb, start=True,
                                 stop=False)
                nc.tensor.matmul(pa[:, 0:384], W['nPd4'][i], W['xkb'],
                                 start=False, stop=True)
                EK = blk.tile([128, 384], BF16, tag="EK")
                nc.scalar.copy(EK, pa[:, 0:384])
                pb = ps_p.tile([128, 512], F32, tag="pp")
                nc.tensor.matmul(pb[:, 0:384], W['Wv'][i], fb, start=True,
                                 stop=False)
                nc.tensor.matmul(pb[:, 0:384], W['nPd4'][i], W['xkb'],
                                 start=False, stop=True)
                VK = blk.tile([128, 384], BF16, tag="VK")
                nc.vector.tensor_copy(VK, pb[:, 0:384])
                pa = ps_h.tile([128, 512], F32, tag="ph")
                nc.tensor.matmul(pa[:, 0:96], W['G1Q'][i], fq, start=True,
                                 stop=False)
                nc.tensor.matmul(pa[:, 0:96], W['G1Pd4'][i], W['xqb'],
                                 start=False, stop=False)
                nc.tensor.matmul(pa[:, 0:96], W['c1'][i], ones96, start=False,
                                 stop=True)
                QB = blk.tile([128, 96], F32, tag="QB")
                nc.scalar.copy(QB, pa[:, 0:96])
                pb = ps_p.tile([128, 512], F32, tag="pp")
                nc.tensor.matmul(pb[:, 0:96], W['Pd4'][i], W['xqb'], start=True,
                                 stop=False)
                nc.tensor.matmul(pb[:, 0:96], W['bpe'][i], ones96, start=False,
                                 stop=True)
                QP = blk.tile([128, 96], F32, tag="QP")
                nc.vector.tensor_copy(QP, pb[:, 0:96])
                return EK, VK, QB, QP

            def warm_pe(n, dep_row):
                # Back-to-back 384-col dummy matmuls keep the tensor engine's
                # p-state ramp alive through a tail bubble so the consts /
                # MLP / next-block matmuls run at full speed. dep_row gates
                # the burst start on a mid-tail tensor (bf16 [1,384] slice).
                for _ in range(n):
                    t = ps_p.tile([128, 512], F32, tag="pp")
                    nc.tensor.matmul(t[:, 0:384], wpack[0:1, 0:128], dep_row,
                                     start=True, stop=True)

            def emit_hpre_for(i, m, EK):
                s4o = 32 * (m % 3)
                t = ps_h.tile([128, 512], F32, tag="ph")
                nc.tensor.matmul(
                    t[:, 0:384],
                    wpack[s4o:s4o + 30, W_G1P + 128 * i: W_G1P + 128 * (i + 1)],
                    tsl_of(m), start=True, stop=False)
                nc.tensor.matmul(t[:, 0:384], W['Wg1'][i], EK,
                                 start=False, stop=True)
                return t

            # block-0 consts hoisted before the trig phase so the first PREQ
            # queries' hpre+relu pre-run in the trig shadow (PE/ACT have idle
            # slack there; Relu shares the Sin activation-table set).
            for ci in range(2):
                emit_trig_pair(ci)
            consts0 = make_consts(0)
            pre_h = {}
            for ci in range(2, NCH // 2):
                emit_trig_pair(ci)
                for m in (2 * (ci - 2), 2 * (ci - 2) + 1):
                    if m < PREQ:
                        hq = emit_hpre_for(0, m, consts0[0])
                        h_t = hp.tile([128, 384], BF16, tag="h")
                        nc.vector.tensor_scalar(
                            out=h_t, in0=hq[:, 0:384],
                            scalar1=consts0[2][:, m:m + 1], scalar2=0.0,
                            op0=OP.add, op1=OP.max)
                        pre_h[m] = h_t
            # Pin the natural_log_exp_and_others table (serves Exp/Relu/
            # Identity/Ln/Copy) once after the Sin phase so no further
            # LoadActFuncSet (1283 ns each) lands on block-tail critical paths.
            nc.scalar.add_instruction(mybir.InstLoadActFuncSet(
                name=nc.get_next_instruction_name(), act_func_set_id=6,
                ins=[], outs=[]))

            def affine_evict(src_ap, sc, b2, shape, dt=BF16, tag="aff", pool=None):
                t = (pool or fpool).tile(shape, dt, tag=tag)
                nc.scalar.activation(out=t, in_=src_ap, func=AF.Identity,
                                     bias=b2, scale=sc)
                return t

            magic = vpack[:, V_MAGIC: V_MAGIC + 1]

            def rsqrt_dve(var_ap, tag):
                # 1/sqrt(var+eps) fully on DVE (shift-seed + 1 Newton step);
                # avoids activations whose table loads (1283 ns each) would
                # land on the block-tail critical path.
                # var >> EPS=1e-5 for this model (unit-variance features),
                # so the +EPS add is dropped: error < 1e-5/var, far below
                # the 2e-2 budget. The Newton 0.5 factor folds into the
                # -0.5 constant of the polish step (one fewer DVE hop).
                y = smalls.tile([128, 1], F32, tag=tag + "y")
                nc.vector.tensor_scalar(out=y.bitcast(U32),
                                        in0=var_ap.bitcast(U32),
                                        scalar1=1, scalar2=None,
                                        op0=OP.logical_shift_right)
                nc.vector.tensor_tensor(out=y.bitcast(U32),
                                        in0=magic.bitcast(U32),
                                        in1=y.bitcast(U32), op=OP.subtract)
                a = smalls.tile([128, 1], F32, tag=tag + "a")
                nc.vector.tensor_tensor(out=a, in0=y, in1=y, op=OP.mult)
                nc.vector.tensor_tensor(out=a, in0=var_ap, in1=a, op=OP.mult)
                nc.vector.tensor_scalar(out=a, in0=a, scalar1=-0.5,
                                        scalar2=1.5, op0=OP.mult, op1=OP.add)
                nc.vector.tensor_tensor(out=y, in0=y, in1=a, op=OP.mult)
                return y

            # per-block fraction of relus on ACT (rest on DVE), tuned to
            # balance the engines: block 0's DVE also carries the trig
            # range-reduction, blocks 1-2 split ~78/22.
            def relu_on_act(i, m):
                if m % 2 == 0:
                    return True
                return ((m // 2) * 9) % 16 < 9

            # ---------- transformer blocks ----------
            for i in range(NB):
                if i == 0:
                    EK, VK, QB, QP = consts0
                else:
                    EK, VK, QB, QP = make_consts(i)

                S1 = blk.tile([128, 96], F32, tag="S1")
                S0 = blk.tile([128, 96], F32, tag="S0")
                payload = blk.tile([128, 96], BF16, tag="payload")
                o_full = wide.tile([128, 768], BF16, tag="ofull")
                sto = smalls.tile([128, 16, 6], F32, tag="sto")

                def gather_part(hs, hw, pt):
                    Rh = smalls.tile([128, hw], F32, tag=f"R{pt}")
                    nc.vector.reciprocal(out=Rh, in_=S0[:, hs:hs + hw])
                    nc.vector.tensor_tensor(out=payload[:, hs:hs + hw],
                                            in0=S1[:, hs:hs + hw], in1=Rh,
                                            op=OP.mult)
                    nc.vector.tensor_tensor(out=payload[:, hs:hs + hw],
                                            in0=payload[:, hs:hs + hw],
                                            in1=fq[:, hs:hs + hw], op=OP.add)
                    ag_in = dram.tile([128, hw], BF16, tag=f"agin{i}h{pt}")
                    if variant == "spmd":
                        ag_out = dram.tile([8, 128, hw], BF16,
                                           addr_space="Shared",
                                           tag=f"agout{i}h{pt}")
                    else:
                        ag_out = dram.tile([8, 128, hw], BF16,
                                           tag=f"agout{i}h{pt}")
                    nc.gpsimd.dma_start(out=ag_in, in_=payload[:, hs:hs + hw])
                    ago_ap = ag_out[:]
                    pay_ap = payload[:]
                    if variant == "spmd":
                        nc.gpsimd.collective_compute(
                            "AllGather", OP.bypass, replica_groups=RG,
                            ins=[ag_in[:].opt()], outs=[ag_out[:].opt()])
                    else:
                        bsrc = bass.AP(tensor=pay_ap.tensor,
                                       offset=pay_ap.offset + hs,
                                       ap=[list(pay_ap.ap[0]), [0, 8],
                                           [1, hw]])
                        bdst = bass.AP(tensor=ago_ap.tensor,
                                       offset=ago_ap.offset,
                                       ap=[[hw, 128], [128 * hw, 8], [1, hw]])
                        nc.sync.dma_start(out=bdst, in_=bsrc)
                    of_ap = o_full[:]
                    odst = bass.AP(tensor=of_ap.tensor,
                                   offset=of_ap.offset + hs,
                                   ap=[list(of_ap.ap[0]), [96, 8], [1, hw]])
                    src = bass.AP(tensor=ago_ap.tensor, offset=ago_ap.offset,
                                  ap=[[hw, 128], [128 * hw, 8], [1, hw]])
                    eng = nc.sync if pt == 0 else nc.scalar
                    eng.dma_start(out=odst, in_=src)
                    # stats over the freshly landed region (84- or 12-wide
                    # slices of each core's 96 columns, one stat group per
                    # core) so only the tiny second gather's stats sit on
                    # the block-tail critical path.
                    for g in range(8):
                        st_v = bass.AP(tensor=of_ap.tensor,
                                       offset=of_ap.offset + hs + 96 * g,
                                       ap=[list(of_ap.ap[0]), [1, hw]])
                        nc.vector.bn_stats(out=sto[:, 8 * pt + g, :], in_=st_v)

                # attention over 96 queries in 48 pairs.
                pre = dict(pre_h) if i == 0 else {}
                nxt = {}
                prime = (PREQ, PREQ + 1) if i == 0 else (0, 1)
                for m in prime:
                    if m not in pre:
                        nxt[m] = emit_hpre_for(i, m, EK)
                pend = []
                w_for = {}
                lg_prev = None

                def drain_one():
                    m = pend.pop(0)
                    w_sl = w_for.pop(m)
                    # S0 on DVE in 4x_2p mode: all-SBUF bf16 -> scratch,
                    # accumulator gives the per-query sum of w.
                    nc.vector.tensor_scalar(out=sdum, in0=w_sl,
                                            scalar1=1.0, scalar2=0.0,
                                            op0=OP.mult, op1=OP.add,
                                            accum_out=S0[:, m:m + 1])
                    pos = ps_p.tile([128, 512], F32, tag="pp")
                    s4o = 32 * (m % 3)
                    nc.tensor.matmul(
                        pos[:, 0:384],
                        wpack[s4o:s4o + 30, W_PS + 128 * i: W_PS + 128 * (i + 1)],
                        tsl_of(m), start=True, stop=False)
                    nc.tensor.matmul(pos[:, 0:384], W['i128'], VK,
                                     start=False, stop=True)
                    nc.vector.scalar_tensor_tensor(
                        out=dumA.broadcast_to((128, 384)),
                        in0=pos[:, 0:384], scalar=QP[:, m:m + 1],
                        in1=w_sl, op0=OP.add, op1=OP.mult,
                        accum_out=S1[:, m:m + 1])
                    if m == 83:
                        gather_part(0, 84, 0)

                def emit_relu(m):
                    if m in pre:
                        return pre.pop(m)
                    hpre = nxt.pop(m)
                    h_t = hp.tile([128, 384], BF16, tag="h")
                    if relu_on_act(i, m):
                        nc.scalar.activation(out=h_t, in_=hpre[:, 0:384],
                                             func=AF.Relu,
                                             bias=QB[:, m:m + 1], scale=1.0)
                    else:
                        nc.vector.tensor_scalar(
                            out=h_t, in0=hpre[:, 0:384],
                            scalar1=QB[:, m:m + 1], scalar2=0.0,
                            op0=OP.add, op1=OP.max)
                    return h_t
                for pr in range(NPAIR):
                    a, b = 2 * pr, 2 * pr + 1
                    h_a = emit_relu(a)
                    lg = ps_l.tile([128, 1024], F32, tag="lg")
                    nc.tensor.matmul(lg[:, 0:384], W['Wg2'][i], h_a,
                                     start=True, stop=True)
                    # exp for the PREVIOUS pair sits BETWEEN the two relus on
                    # ACT: its logits completed an iteration ago (no stall),
                    # it finishes early enough that the next iteration's lg
                    # matmuls never WAR-wait on it, and relu(b) stays early
                    # enough that lg(b) beats the exp that needs it next
                    # iteration. s0/stt drain one further pair behind.
                    if lg_prev is not None:
                        w_pair = wp.tile([128, 768], BF16, tag="w")
                        nc.scalar.activation(out=pair_view(w_pair[:], 384),
                                             in_=pair_view(lg_prev[:], 512),
                                             func=AF.Exp, bias=W['bg2'][i],
                                             scale=1.0)
                        w_for[a - 2] = w_pair[:, 0:384]
                        w_for[b - 2] = w_pair[:, 384:768]
                        pend.append(a - 2)
                        pend.append(b - 2)
                    lg_prev = lg
                    h_b = emit_relu(b)
                    nc.tensor.matmul(lg[:, 512:896], W['Wg2'][i], h_b,
                                     start=True, stop=True)
                    if pr == NPAIR - 1:
                        # final pair: collapse the software-pipeline lag so
                        # the flush is just the last two drains.
                        w_pair = wp.tile([128, 768], BF16, tag="w")
                        nc.scalar.activation(out=pair_view(w_pair[:], 384),
                                             in_=pair_view(lg[:], 512),
                                             func=AF.Exp, bias=W['bg2'][i],
                                             scale=1.0)
                        w_for[a] = w_pair[:, 0:384]
                        w_for[b] = w_pair[:, 384:768]
                        pend.append(a)
                        pend.append(b)
                        while pend:
                            drain_one()
                    else:
                        while len(pend) > 2:
                            drain_one()
                    for m in (a + 2, b + 2):
                        if m < NQ and m not in pre and m not in nxt:
                            nxt[m] = emit_hpre_for(i, m, EK)
                gather_part(84, 12, 1)
                warm_pe(40 if i == 0 else 12, w_pair[0:1, 0:384])

                # block tail via half-gathers: queries 0-83 are reduced,
                # all-gathered, written back and stats'd while 84-95 still
                # compute; only the 12-query gather's stats remain here.
                mv = smalls.tile([128, 2], F32, tag="bnmv")
                nc.vector.bn_aggr(out=mv, in_=sto)
                rs = rsqrt_dve(mv[:, 1:2], "bn")
                sc = smalls.tile([128, 1], F32, tag="sc")
                nc.vector.tensor_tensor(out=sc, in0=W['gam'][i], in1=rs, op=OP.mult)
                b2 = smalls.tile([128, 1], F32, tag="b2")
                nc.vector.tensor_scalar(out=b2, in0=mv[:, 0:1], scalar1=sc,
                                        scalar2=None, op0=OP.mult)
                nc.vector.tensor_tensor(out=b2, in0=W['bet'][i], in1=b2, op=OP.subtract)

                if i < NB - 1:
                    fq = fpool.tile([128, 96], BF16, tag="fq")
                    nc.vector.tensor_scalar(out=fq, in0=payload[:, 0:96],
                                            scalar1=sc, scalar2=b2,
                                            op0=OP.mult, op1=OP.add)
                if i == 0:
                    fb = fpool.tile([128, 384], BF16, tag="fb")
                    with tc.If(pid < 4) as cmp:
                        nc.scalar.activation(out=fb, in_=o_full[:, 0:384],
                                             func=AF.Identity, bias=b2, scale=sc)
                    with cmp.Else():
                        nc.scalar.activation(out=fb, in_=o_full[:, 384:768],
                                             func=AF.Identity, bias=b2, scale=sc)

                # ---------- MLP ----------
                if i > 0:
                    j = i - 1

                    def mlp_wide(lw, bias_ap, xin, tag):
                        # 768-wide layer: 2 matmul banks + ONE pair relu
                        t = wide.tile([128, 768], BF16, tag=tag)
                        pp = ps_l.tile([128, 1024], F32, tag="lg")
                        nc.tensor.matmul(pp[:, 0:384], lw, xin[:, 0:384],
                                         start=True, stop=True)
                        nc.tensor.matmul(pp[:, 512:896], lw, xin[:, 384:768],
                                         start=True, stop=True)
                        nc.scalar.activation(out=pair_view(t[:], 384),
                                             in_=pair_view(pp[:], 512),
                                             func=AF.Relu, bias=bias_ap,
                                             scale=1.0)
                        return t

                    def mlp_q(lw, bias_ap, xin, tag):
                        t = wide.tile([128, 96], BF16, tag=tag)
                        pp = ps_h.tile([128, 512], F32, tag="ph")
                        nc.tensor.matmul(pp[:, 0:96], lw, xin, start=True,
                                         stop=True)
                        nc.scalar.activation(out=t, in_=pp[:, 0:96],
                                             func=AF.Relu, bias=bias_ap,
                                             scale=1.0)
                        return t

                    # fold the BN scale into em1 (per input channel =
                    # per partition of the transposed weights) so the MLP
                    # consumes the gathered bf16 o_full directly -- the
                    # f_full eviction leaves the tail critical path. The
                    # per-channel BN shift b2 is folded into the layer-1
                    # bias; it is NOT added to the residual (o2 = sc*o+y2),
                    # which cancels exactly through the next BatchNorm's own
                    # mean subtraction.
                    em1s = smalls.tile([128, 128], BF16, tag="em1s")
                    nc.vector.tensor_scalar(out=em1s, in0=W['em1'][j],
                                            scalar1=sc, scalar2=None,
                                            op0=OP.mult)
                    b2bf = smalls.tile([128, 1], BF16, tag="b2bf")
                    nc.vector.tensor_scalar(out=b2bf, in0=b2, scalar1=1.0,
                                            scalar2=None, op0=OP.mult)
                    pbias = ps_h.tile([128, 512], F32, tag="ph")
                    nc.tensor.matmul(pbias[:, 0:1], W['em1'][j], b2bf,
                                     start=True, stop=True)
                    bias1 = smalls.tile([128, 1], F32, tag="bias1")
                    nc.vector.tensor_tensor(out=bias1, in0=pbias[:, 0:1],
                                            in1=W['emb1'][j], op=OP.add)
                    y1f = mlp_wide(em1s, bias1, o_full, "y1f")
                    y2f = mlp_wide(W['em2'][j], W['emb2'][j], y1f, "y2f")
                    o2f = wide.tile([128, 768], BF16, tag="o2f")
                    o_s = wide.tile([128, 768], BF16, tag="o_s")
                    nc.vector.tensor_scalar(out=o_s, in0=o_full, scalar1=sc,
                                            scalar2=None, op0=OP.mult)
                    nc.vector.tensor_tensor(out=o2f, in0=o_s, in1=y2f,
                                            op=OP.add)
                    warm_pe(22, o2f[0:1, 0:384])
                    if i < NB - 1:
                        y1q = mlp_q(W['em1'][j], W['emb1'][j], fq, "y1q")
                        y2q = mlp_q(W['em2'][j], W['emb2'][j], y1q, "y2q")
                        o2q = wide.tile([128, 96], BF16, tag="o2q")
                        nc.vector.scalar_tensor_tensor(
                            out=o2q, in0=payload[:, 0:96], scalar=sc,
                            in1=y2q, op0=OP.mult, op1=OP.add)

                    st2 = smalls.tile([128, 2, 6], F32, tag="st2")
                    nc.vector.bn_stats(out=st2[:, 0, :], in_=o2f[:, 0:384])
                    nc.vector.bn_stats(out=st2[:, 1, :], in_=o2f[:, 384:768])
                    mv2 = smalls.tile([128, 2], F32, tag="mv2")
                    nc.vector.bn_aggr(out=mv2, in_=st2)
                    rs2 = rsqrt_dve(mv2[:, 1:2], "em")
                    sc2 = smalls.tile([128, 1], F32, tag="sc")
                    nc.vector.tensor_tensor(out=sc2, in0=W['emg'][j], in1=rs2,
                                            op=OP.mult)
                    b22 = smalls.tile([128, 1], F32, tag="b2")
                    nc.vector.tensor_scalar(out=b22, in0=mv2[:, 0:1], scalar1=sc2,
                                            scalar2=None, op0=OP.mult)
                    nc.vector.tensor_tensor(out=b22, in0=W['embe'][j], in1=b22,
                                            op=OP.subtract)
                    if i == NB - 1:
                        f_full = fpool.tile([128, 768], BF16, tag="ffull")
                        nc.scalar.activation(out=f_full[:, 0:384],
                                             in_=o2f[:, 0:384],
                                             func=AF.Identity, bias=b22,
                                             scale=sc2)
                        nc.scalar.activation(out=f_full[:, 384:768],
                                             in_=o2f[:, 384:768],
                                             func=AF.Identity, bias=b22,
                                             scale=sc2)
                    if i < NB - 1:
                        fb = fpool.tile([128, 384], BF16, tag="fb")
                        with tc.If(pid < 4) as cmp:
                            nc.scalar.activation(out=fb, in_=o2f[:, 0:384],
                                                 func=AF.Identity, bias=b22,
                                                 scale=sc2)
                        with cmp.Else():
                            nc.scalar.activation(out=fb, in_=o2f[:, 384:768],
                                                 func=AF.Identity, bias=b22,
                                                 scale=sc2)
                        fq = fpool.tile([128, 96], BF16, tag="fq")
                        nc.vector.tensor_scalar(out=fq, in0=o2q, scalar1=sc2,
                                                scalar2=b22, op0=OP.mult,
                                                op1=OP.add)

            # ---------- final FC + max ----------
            ot4 = sing.tile([128, 4], F32, tag="ot4")
            for bb in range(2):
                fbb = f_full[:, bb * 384:(bb + 1) * 384]
                pp = ps_l.tile([128, 1024], F32, tag="lg")
                nc.tensor.matmul(pp[:, 0:384], W['f1'][0], fbb, start=True,
                                 stop=True)
                nc.tensor.matmul(pp[:, 512:896], W['f1'][1], fbb, start=True,
                                 stop=True)
                e1 = wide.tile([128, 768], BF16, tag="e1")
                e1_v = bass.AP(tensor=e1[:].tensor, offset=e1[:].offset,
                               ap=[list(e1[:].ap[0]), [384, 2], [1, 384]])
                # per-half bias differs (f1b[0]/f1b[1]); pair relu needs one
                # bias, so split the relu in two over the wide psum tile.
                nc.scalar.activation(out=e1[:, 0:384], in_=pp[:, 0:384],
                                     func=AF.Relu, bias=W['f1b'][0], scale=1.0)
                nc.vector.tensor_scalar(out=e1[:, 384:768], in0=pp[:, 512:896],
                                        scalar1=W['f1b'][1], scalar2=0.0,
                                        op0=OP.add, op1=OP.max)
                pq = ps_l.tile([128, 1024], F32, tag="lg")
                for h in range(2):
                    nc.tensor.matmul(pq[:, 512 * h: 512 * h + 384],
                                     W['f2'][h][0], e1[:, 0:384],
                                     start=True, stop=False)
                    nc.tensor.matmul(pq[:, 512 * h: 512 * h + 384],
                                     W['f2'][h][1], e1[:, 384:768],
                                     start=False, stop=True)
                mx = smalls.tile([128, 2], F32, tag="mx")
                nc.vector.tensor_reduce(out=mx, in_=pair_view(pq[:], 512),
                                        axis=mybir.AxisListType.X, op=OP.max)
                for h in range(2):
                    nc.vector.tensor_scalar(out=ot4[:, 2 * bb + h: 2 * bb + h + 1],
                                            in0=mx[:, h:h + 1], scalar1=W['f2b'][h],
                                            scalar2=None, op0=OP.add)
                od_ap = out_d[:]
                odst = bass.AP(tensor=od_ap.tensor,
                               offset=od_ap.offset + 256 * bb,
                               ap=[[1, 128], [128, 2]])
                nc.sync.dma_start(out=odst, in_=ot4[:, 2 * bb: 2 * bb + 2])

    nc.compile()
    _CACHE[variant] = nc
    return nc


def _prep_inputs(inputs):
    """Host-side constant relayout + per-core packing. Returns in_maps list."""
    xyz = _f32(inputs["xyz"])          # [2, 384, 3]
    feats = _f32(inputs["feats"])      # [2, 384, 1]

    Wq, Wk, Wv = inputs["tb_Wq"], inputs["tb_Wk"], inputs["tb_Wv"]
    Wg1, bg1 = inputs["tb_Wg1"], inputs["tb_bg1"]
    Wg2, bg2 = inputs["tb_Wg2"], inputs["tb_bg2"]
    Wpe, bpe = inputs["tb_Wpe"], inputs["tb_bpe"]

    wpack = np.zeros((128, WCOLS), np.float32)
    vpack = np.zeros((128, VCOLS), np.float32)
    rpack_c = np.zeros((3, RCOLS), np.float32)   # per-core cols filled later
    qpack_c = np.zeros((1, QCOLS), np.float32)

    for i in range(NB):
        Ws, Wd = _wpe_split(_f32(Wpe[i]))
        g1 = _f32(Wg1[i])
        wpack[:, W_WG1 + 128 * i: W_WG1 + 128 * (i + 1)] = g1.T
        wpack[:, W_WG2 + 128 * i: W_WG2 + 128 * (i + 1)] = _f32(Wg2[i]).T
        wpack[:, W_NWK + 128 * i: W_NWK + 128 * (i + 1)] = (-_f32(Wk[i])).T
        wpack[:, W_WV + 128 * i: W_WV + 128 * (i + 1)] = _f32(Wv[i]).T
        wpack[:, W_G1Q + 128 * i: W_G1Q + 128 * (i + 1)] = (g1 @ _f32(Wq[i])).T
        wpack[:, W_G1P + 128 * i: W_G1P + 128 * (i + 1)] = _rep3((g1 @ Ws).T)
        wpack[:, W_PS + 128 * i: W_PS + 128 * (i + 1)] = _rep3(Ws.T)
        rpack_c[:, R_NPD4 + 128 * i: R_NPD4 + 128 * (i + 1)] = (-4.0 * Wd).T
        rpack_c[:, R_G1PD4 + 128 * i: R_G1PD4 + 128 * (i + 1)] = (4.0 * (g1 @ Wd)).T
        rpack_c[:, R_PD4 + 128 * i: R_PD4 + 128 * (i + 1)] = (4.0 * Wd).T
        qpack_c[0, Q_C1 + 128 * i: Q_C1 + 128 * (i + 1)] = g1 @ _f32(bpe[i]) + _f32(bg1[i])
        qpack_c[0, Q_BPE + 128 * i: Q_BPE + 128 * (i + 1)] = _f32(bpe[i])
        vpack[:, V_BG2 + i] = _f32(bg2[i])
        vpack[:, V_GAM + i] = _f32(inputs["tb_gamma"][i])
        vpack[:, V_BET + i] = _f32(inputs["tb_beta"][i])

    wpack[:, W_I128: W_I128 + 128] = np.eye(128, dtype=np.float32)
    for j in range(NF):
        wpack[:, W_EM1 + 128 * j: W_EM1 + 128 * (j + 1)] = _f32(inputs["em_W1"][j]).T
        wpack[:, W_EM2 + 128 * j: W_EM2 + 128 * (j + 1)] = _f32(inputs["em_W2"][j]).T
        vpack[:, V_EMB1 + j] = _f32(inputs["em_b1"][j])
        vpack[:, V_EMB2 + j] = _f32(inputs["em_b2"][j])
        vpack[:, V_EMG + j] = _f32(inputs["em_gamma"][j])
        vpack[:, V_EMBE + j] = _f32(inputs["em_beta"][j])
    W1T = _f32(inputs["fcf_W1"]).T           # [128, 256]
    for h in range(2):
        wpack[:, W_F1 + 128 * h: W_F1 + 128 * (h + 1)] = W1T[:, h * 128:(h + 1) * 128]
        vpack[:, V_F1B + h] = _f32(inputs["fcf_b1"])[h * 128:(h + 1) * 128]
        vpack[:, V_F2B + h] = _f32(inputs["fcf_b2"])[h * 128:(h + 1) * 128]
    W2T = _f32(inputs["fcf_W2"]).T           # [256, 256]
    for h in range(2):
        for k in range(2):
            wpack[:, W_F2 + 128 * (2 * h + k): W_F2 + 128 * (2 * h + k + 1)] = \
                W2T[k * 128:(k + 1) * 128, h * 128:(h + 1) * 128]
    vpack[:, V_ENCB] = _f32(inputs["enc_b"])
    vpack[:, V_MAGIC] = np.array([0x5F3759DF], np.uint32).view(np.float32)[0]
    qpack_c[0, Q_FEATS: Q_FEATS + 768] = feats.reshape(768)
    qpack_c[0, Q_ENC: Q_ENC + 128] = _f32(inputs["enc_W"])[:, 0]

    wpack_b = _bf(wpack)
    vpack_f = _f32(vpack)

    # s coefficients: r = s*xk - (s*xq - off), s = 4*freq/2pi
    svals = (4.0 * FREQS / TWO_PI)  # [5] f64

    in_maps = []
    for c in range(8):
        b, qo = c // 4, (c % 4) * 96
        xk = xyz[b].T                      # [3, 384]

        # fpack: bf16 split-precision trig matmul operands.
        # rhs rows: 0-14 z_hi (5j+f), 15-29 z_lo, 30-31 ones.
        xko = np.zeros((32, 384), np.float32)
        for j in range(3):
            for f in range(5):
                zv = svals[f] * xyz[b, :, j].astype(np.float64)  # [384]
                zh = np.float32(np.asarray(zv, np.float32).astype(BF))
                zl = (zv - zh).astype(np.float32).astype(BF)
                xko[5 * j + f] = zh
                xko[15 + 5 * j + f] = np.float32(zl)
        xko[30] = 1.0
        xko[31] = 1.0

        # lhsT: selector rows pair z_hi and z_lo; c rows carry the
        # per-query constant split hi/lo.
        S4 = np.zeros((32, NCH * 96), np.float32)
        for cch in range(NCH):
            for s in range(3):
                qg = qo + 3 * cch + s
                for j in range(3):
                    for t in range(10):
                        col = 96 * cch + 32 * s + 10 * j + t
                        f = t % 5
                        S4[5 * j + f, col] = 1.0
                        S4[15 + 5 * j + f, col] = 1.0
                        cval = (0.25 if t >= 5 else 0.0) - \
                            svals[f] * np.float64(xyz[b, qg, j])
                        chv = np.float32(np.asarray(cval, np.float32).astype(BF))
                        clv = np.float32(np.float32(cval - chv).astype(BF))
                        S4[30, col] = chv
                        S4[31, col] = clv
        fpack = np.zeros((32, FCOLS), np.float32)
        fpack[:, F_S4: F_S4 + NCH * 96] = S4
        fpack[:, F_XKO: F_XKO + 384] = xko

        rpack = rpack_c.copy()
        rpack[:, R_XKB: R_XKB + 384] = xk
        rpack[:, R_XQB: R_XQB + 96] = xk[:, qo:qo + 96]
        qpack = qpack_c.copy()
        qpack[0, Q_FB: Q_FB + 384] = feats[b].reshape(384)
        qpack[0, Q_FQ: Q_FQ + 96] = feats[b, qo:qo + 96].reshape(96)

        in_maps.append({
            "wpack": wpack_b,
            "vpack": vpack_f,
            "rpack": _bf(rpack),
            "qpack": _bf(qpack),
            "fpack": _bf(fpack),
        })
    return in_maps


def kernel(**inputs):
    from concourse.bass_utils import run_bass_kernel_spmd

    nc = _build()
    in_maps = _prep_inputs(inputs)
    res = run_bass_kernel_spmd(nc, in_maps, list(range(8)))
    return np.asarray(res.results[0]["out"], np.float32)


if __name__ == "__main__":
    rng = np.random.RandomState(0)
    fake = {
        "xyz": rng.randn(2, 384, 3).astype(np.float32),
        "feats": rng.randn(2, 384, 1).astype(np.float32),
    }
    print("smoke build only")


# revision 49
# speedup vs baseline: 1.0004x; 1.0004x over previous
"""NePuEncoder Bass/Tile kernel for 8 Trainium2 NeuronCores.

Sharding: query-parallel. Core c handles batch b=c//4, queries qo=(c%4)*96 ..
qo+96 of that batch. Channel-major layout [128 chan, keys] throughout.

Per-channel softmax attention fully fused in SBUF/PSUM:
  - pairwise trig features via range-reduced Sin (|arg|<=pi): 32 chunks of 3
    queries, computed once and reused across all 3 transformer blocks. The
    r = s*(xk-xq)+off matmul runs in bf16 via split-precision (z_hi/z_lo +
    c_hi/c_lo rhs rows against 0/1-selector lhsT columns, K=32) instead of
    fp32 (4 cyc/row). Range reduction is split across engines: round-to-int
    on ACT (Identity + C_ROUND bias, f32 store), -frac via one DVE
    scalar_tensor_tensor, Sin with scale=-2pi; two chunks per instruction
    over 2-bank PSUM tiles. First-24-query relus of block 0 pre-run on DVE
    in this phase's shadow.
  - main loop in query PAIRS with a software-pipelined two-stage lag: at
    iteration p the relus+logit matmuls of pair p are emitted, the Exp of
    pair p-1 (one [128,2,384] strided activation over both PSUM banks, no
    accum_out), and the s0/stt drain of pair p-2 -- so neither ACT nor DVE
    ever head-of-line blocks on an in-flight cross-engine producer. The lag
    collapses at the final pair so the flush is just the last drains.
  - per query, S0 comes from a DVE tensor_scalar in 4x_2p mode (all-SBUF
    bf16 w -> scratch, accum_out=S0, ~160ns) instead of the 505+187ns
    unpaired exp+accumulator; S1 via DVE scalar_tensor_tensor (pos+QP)*w.
    Relu (per-query bias blocks pairing) alternates ACT/DVE: even queries
    on ACT, odd queries 9/16 on ACT, balancing all three engines at
    ~1.65us/pair.
  - PSUM: 8 banks as lg-pairs 2x[128,1024] + hpre ring 2x[128,512] + pos
    ring 2x[128,512]; trig pairs, MLP pairs and the final FC reuse the lg
    pool; make_consts reuses the hpre/pos rings.
  - two 8-rank AllGathers per block (query-halves): queries 0-83 are
    reduced, gathered, written back AND per-core bn_stats'd while 84-95
    still compute, so only the 12-query gather + stats sit on the block
    tail; BatchNorm rsqrt is a DVE-only shift-seed Newton iteration, and a
    single LoadActFuncSet pin keeps one table resident after the Sin phase.
    MLP blocks consume the gathered bf16 o_full directly: the BN scale is
    folded into em1 per input channel (63ns DVE op) and the BN shift into
    the layer-1 bias, while the residual is o2 = sc*o + y2 -- the dropped
    per-channel shift cancels exactly through the next BatchNorm's mean
    subtraction, so the 825ns f_full eviction leaves the tail critical
    path. Payload/gather/o_full are bf16 (half the collective bytes). The
    final FC (pair-wide psum, one 2-group max reduce) and MLPs are
    computed redundantly per core; core 0's output is returned.
  - warm_pe: dep-gated back-to-back dummy 384-col matmuls keep the tensor
    engine's p-state ramp alive through tail bubbles so consts / FC /
    next-block matmuls run at 1 cyc/row instead of the 2-3.7x post-idle
    penalty.

All constant tensors are packed host-side into 5 DRAM tensors loaded with 5
DMAs (split across SP/ACT/Pool queues) at program start.
"""
import sys

sys.path.insert(0, "/opt/trn_rl_repo")

import numpy as np
import ml_dtypes

B, N, D, DS, LAT, FD, NF = 2, 384, 128, 3, 256, 1, 2
NB = NF + 1
NQ = 96                  # queries per core
NCH = 32                 # trig chunks (3 queries each)
NPAIR = NQ // 2
FREQS = np.linspace(1.0, 32.0, 5).astype(np.float64)
EPS = 1e-5
TWO_PI = float(2 * np.pi)
C_ROUND = float(3 << 22)  # 2^23 + 2^22: fp32 round-to-nearest-even trick
PREQ = 24                # block-0 queries whose relu pre-runs in trig shadow

BF = ml_dtypes.bfloat16

# --- wpack column offsets (bf16 [128, WCOLS]) ---
W_WG1, W_WG2, W_NWK, W_WV, W_G1Q = 0, 384, 768, 1152, 1536
W_G1P, W_PS, W_I128 = 1920, 2304, 2688
W_EM1, W_EM2, W_F1, W_F2 = 2816, 3072, 3328, 3584
WCOLS = 4096
# --- vpack column offsets (f32 [128, VCOLS]) ---
V_BG2, V_GAM, V_BET, V_ENCB = 0, 3, 6, 9
V_EMB1, V_EMB2, V_EMG, V_EMBE, V_F1B, V_F2B = 10, 12, 14, 16, 18, 20
V_MAGIC = 22
VCOLS = 23
# --- rpack column offsets (bf16 [3, RCOLS]) ---
R_XKB, R_NPD4, R_G1PD4, R_PD4, R_XQB = 0, 384, 768, 1152, 1536
RCOLS = 1632
# --- qpack column offsets (bf16 [1, QCOLS]) ---
Q_FEATS, Q_ENC, Q_C1, Q_BPE, Q_FB, Q_FQ = 0, 768, 896, 1280, 1664, 2048
QCOLS = 2144
# --- fpack column offsets (bf16 [32, FCOLS]) ---
F_S4, F_XKO = 0, NCH * 96
FCOLS = NCH * 96 + 384


def _bf(x):
    return np.ascontiguousarray(np.asarray(x, np.float32).astype(BF))


def _f32(x):
    return np.ascontiguousarray(np.asarray(x, np.float32))


def _wpe_split(Wpe):
    """W_s [128,30] trig cols (sin-sign absorbed for npd = xk - xq), W_d [128,3]."""
    Ws = np.zeros((D, 30), np.float32)
    for j in range(3):
        for t in range(10):
            r = 10 * j + t
            if t < 5:
                Ws[:, r] = -Wpe[:, 3 + 6 * t + j]
            else:
                Ws[:, r] = Wpe[:, 3 + 6 * (t - 5) + 3 + j]
    return Ws, Wpe[:, 0:3].astype(np.float32)


def _rep3(M30):
    """[30,128] -> [128,128] with copies at partition offsets 0/32/64."""
    out = np.zeros((128, 128), np.float32)
    for s in range(3):
        out[32 * s:32 * s + 30, :] = M30
    return out


_CACHE = {}


def _build(variant="spmd"):
    if variant in _CACHE:
        return _CACHE[variant]

    import concourse.bacc as bacc
    import concourse.bass as bass
    import concourse.tile as tile
    from concourse import mybir

    F32, BF16, U32 = mybir.dt.float32, mybir.dt.bfloat16, mybir.dt.uint32
    AF = mybir.ActivationFunctionType
    OP = mybir.AluOpType

    nc = bacc.Bacc(None, target_bir_lowering=False,
                   num_devices=(8 if variant == "spmd" else 1))

    wpack_d = nc.dram_tensor("wpack", [128, WCOLS], BF16, kind="ExternalInput")
    vpack_d = nc.dram_tensor("vpack", [128, VCOLS], F32, kind="ExternalInput")
    rpack_d = nc.dram_tensor("rpack", [3, RCOLS], BF16, kind="ExternalInput")
    qpack_d = nc.dram_tensor("qpack", [1, QCOLS], BF16, kind="ExternalInput")
    fpack_d = nc.dram_tensor("fpack", [32, FCOLS], BF16, kind="ExternalInput")

    out_d = nc.dram_tensor("out", [2, 256], F32, kind="ExternalOutput")
    RG = [[0, 1, 2, 3, 4, 5, 6, 7]]

    def pair_view(t_ap, half_stride, width=384):
        """[128, 2, width] strided view of an AP (two halves at half_stride)."""
        return bass.AP(tensor=t_ap.tensor, offset=t_ap.offset,
                       ap=[list(t_ap.ap[0]), [half_stride, 2], [1, width]])

    with tile.TileContext(nc) as tc:
        with (
            tc.tile_pool(name="sing", bufs=1) as sing,
            tc.tile_pool(name="fpool", bufs=2) as fpool,
            tc.tile_pool(name="blk", bufs=2) as blk,
            tc.tile_pool(name="hp", bufs=30) as hp,
            tc.tile_pool(name="wp", bufs=3) as wp,
            tc.tile_pool(name="wide", bufs=2) as wide,
            tc.tile_pool(name="smalls", bufs=4) as smalls,
            tc.tile_pool(name="st1", bufs=3) as st1,
            tc.tile_pool(name="ps_l", bufs=2, space="PSUM") as ps_l,
            tc.tile_pool(name="ps_h", bufs=2, space="PSUM") as ps_h,
            tc.tile_pool(name="ps_p", bufs=2, space="PSUM") as ps_p,
            tc.tile_pool(name="dram", bufs=1, space="DRAM") as dram,
        ):
            # ---------- packed constant loads: 5 DMAs, issued first ----------
            fpack = sing.tile([32, FCOLS], BF16, tag="fpack", name="fpack")
            nc.sync.dma_start(out=fpack, in_=fpack_d[:])
            wpack = sing.tile([128, WCOLS], BF16, tag="wpack", name="wpack")
            nc.gpsimd.dma_start(out=wpack, in_=wpack_d[:])
            vpack = sing.tile([128, VCOLS], F32, tag="vpack", name="vpack")
            nc.scalar.dma_start(out=vpack, in_=vpack_d[:])
            rpack = sing.tile([3, RCOLS], BF16, tag="rpack", name="rpack")
            nc.scalar.dma_start(out=rpack, in_=rpack_d[:])
            qpack = sing.tile([1, QCOLS], BF16, tag="qpack", name="qpack")
            nc.sync.dma_start(out=qpack, in_=qpack_d[:])

            def wsl(off, i=0):
                return wpack[:, off + 128 * i: off + 128 * (i + 1)]

            W = {
                "Wg1": [wsl(W_WG1, i) for i in range(NB)],
                "Wg2": [wsl(W_WG2, i) for i in range(NB)],
                "nWk": [wsl(W_NWK, i) for i in range(NB)],
                "Wv": [wsl(W_WV, i) for i in range(NB)],
                "G1Q": [wsl(W_G1Q, i) for i in range(NB)],
                "i128": wsl(W_I128),
                "em1": [wsl(W_EM1, j) for j in range(NF)],
                "em2": [wsl(W_EM2, j) for j in range(NF)],
                "f1": [wsl(W_F1, h) for h in range(2)],
                "f2": [[wsl(W_F2, 2 * h + k) for k in range(2)] for h in range(2)],
                "bg2": [vpack[:, V_BG2 + i: V_BG2 + i + 1] for i in range(NB)],
                "gam": [vpack[:, V_GAM + i: V_GAM + i + 1] for i in range(NB)],
                "bet": [vpack[:, V_BET + i: V_BET + i + 1] for i in range(NB)],
                "encb": vpack[:, V_ENCB: V_ENCB + 1],
                "emb1": [vpack[:, V_EMB1 + j: V_EMB1 + j + 1] for j in range(NF)],
                "emb2": [vpack[:, V_EMB2 + j: V_EMB2 + j + 1] for j in range(NF)],
                "emg": [vpack[:, V_EMG + j: V_EMG + j + 1] for j in range(NF)],
                "embe": [vpack[:, V_EMBE + j: V_EMBE + j + 1] for j in range(NF)],
                "f1b": [vpack[:, V_F1B + h: V_F1B + h + 1] for h in range(2)],
                "f2b": [vpack[:, V_F2B + h: V_F2B + h + 1] for h in range(2)],
                "xkb": rpack[:, R_XKB: R_XKB + 384],
                "xqb": rpack[:, R_XQB: R_XQB + 96],
                "nPd4": [rpack[:, R_NPD4 + 128 * i: R_NPD4 + 128 * (i + 1)] for i in range(NB)],
                "G1Pd4": [rpack[:, R_G1PD4 + 128 * i: R_G1PD4 + 128 * (i + 1)] for i in range(NB)],
                "Pd4": [rpack[:, R_PD4 + 128 * i: R_PD4 + 128 * (i + 1)] for i in range(NB)],
                "feats": qpack[:, Q_FEATS: Q_FEATS + 768],
                "enc": qpack[:, Q_ENC: Q_ENC + 128],
                "c1": [qpack[:, Q_C1 + 128 * i: Q_C1 + 128 * (i + 1)] for i in range(NB)],
                "bpe": [qpack[:, Q_BPE + 128 * i: Q_BPE + 128 * (i + 1)] for i in range(NB)],
                "featsb": qpack[:, Q_FB: Q_FB + 384],
                "featsq": qpack[:, Q_FQ: Q_FQ + 96],
            }

            ones96 = sing.tile([1, 96], BF16, tag="ones96")
            nc.vector.memset(ones96, 1.0)
            crt = sing.tile([128, 1], F32, tag="crt")
            nc.vector.memset(crt, C_ROUND)
            ncrt = sing.tile([128, 1], F32, tag="ncrt")
            nc.vector.memset(ncrt, -C_ROUND)
            dumA = sing.tile([128, 1], BF16, tag="dumA")
            sdum = sing.tile([128, 384], BF16, tag="sdum")

            # 128 partitions so the tile is forced to base partition 0 (PE
            # lhsT/rhs base must be 0/32/64); rows 96-127 unused.
            trigc = sing.tile([128, NCH * 384], BF16, tag="trig", name="trig")

            def tsl_of(m):
                s4o = 32 * (m % 3)
                c = m // 3
                return trigc[s4o:s4o + 30, 384 * c: 384 * c + 384]

            # ---------- stage 1: trig features, bf16 split-precision ----------
            # r[row,k] = z_hi + z_lo (+ c_hi + c_lo) summed via one bf16
            # matmul (K=32); n = (r+C)-C (round); -frac = (n-C)-r handled as
            # nf = r - n; trig = sin(2pi*nf) -> bf16. Two chunks (8 queries)
            # per range-reduction / Sin instruction.
            xko = fpack[:, F_XKO: F_XKO + 384]

            def emit_trig_pair(ci):
                c0, c1 = 2 * ci, 2 * ci + 1
                rp = ps_l.tile([128, 1024], F32, tag="lg")
                nc.tensor.matmul(rp[0:96, 0:384],
                                 fpack[:, F_S4 + 96 * c0: F_S4 + 96 * c0 + 96],
                                 xko, start=True, stop=True)
                nc.tensor.matmul(rp[0:96, 512:896],
                                 fpack[:, F_S4 + 96 * c1: F_S4 + 96 * c1 + 96],
                                 xko, start=True, stop=True)
                rp96 = rp[0:96, :]
                rp_v = pair_view(rp96, 512)
                rpc = st1.tile([96, 768], F32, tag="nt")
                nc.scalar.activation(out=pair_view(rpc[:], 384), in_=rp_v,
                                     func=AF.Identity, bias=crt[0:96, 0:1],
                                     scale=1.0)
                nf = st1.tile([96, 768], F32, tag="nf")
                nc.vector.scalar_tensor_tensor(
                    out=pair_view(nf[:], 384), in0=pair_view(rpc[:], 384),
                    scalar=ncrt[0:96, 0:1], in1=rp_v, op0=OP.add,
                    op1=OP.subtract)
                nc.scalar.activation(out=trigc[0:96, 384 * c0: 384 * c0 + 768],
                                     in_=nf[:, 0:768], func=AF.Sin,
                                     bias=0.0, scale=-TWO_PI)

            # ---------- initial features ----------
            fb = fpool.tile([128, 384], BF16, tag="fb")
            p = ps_h.tile([128, 512], F32, tag="ph")
            nc.tensor.matmul(p[:, 0:384], W['enc'], W['featsb'], start=True, stop=True)
            nc.scalar.activation(out=fb, in_=p[:, 0:384], func=AF.Identity,
                                 bias=W['encb'], scale=1.0)
            fq = fpool.tile([128, 96], BF16, tag="fq")
            p = ps_h.tile([128, 512], F32, tag="ph")
            nc.tensor.matmul(p[:, 0:96], W['enc'], W['featsq'], start=True, stop=True)
            nc.scalar.activation(out=fq, in_=p[:, 0:96], func=AF.Identity,
                                 bias=W['encb'], scale=1.0)

            pid = nc.scalar.partition_id()

            def make_consts(i):
                pa = ps_h.tile([128, 512], F32, tag="ph")
                nc.tensor.matmul(pa[:, 0:384], W['nWk'][i], fb, start=True,
                                 stop=False)
                nc.tensor.matmul(pa[:, 0:384], W['nPd4'][i], W['xkb'],
                                 start=False, stop=True)
                EK = blk.tile([128, 384], BF16, tag="EK")
                nc.scalar.copy(EK, pa[:, 0:384])
                pb = ps_p.tile([128, 512], F32, tag="pp")
                nc.tensor.matmul(pb[:, 0:384], W['Wv'][i], fb, start=True,
                                 stop=False)
                nc.tensor.matmul(pb[:, 0:384], W['nPd4'][i], W['xkb'],
                                 start=False, stop=True)
                VK = blk.tile([128, 384], BF16, tag="VK")
                nc.vector.tensor_copy(VK, pb[:, 0:384])
                pa = ps_h.tile([128, 512], F32, tag="ph")
                nc.tensor.matmul(pa[:, 0:96], W['G1Q'][i], fq, start=True,
                                 stop=False)
                nc.tensor.matmul(pa[:, 0:96], W['G1Pd4'][i], W['xqb'],
                                 start=False, stop=False)
                nc.tensor.matmul(pa[:, 0:96], W['c1'][i], ones96, start=False,
                                 stop=True)
                QB = blk.tile([128, 96], F32, tag="QB")
                nc.scalar.copy(QB, pa[:, 0:96])
                pb = ps_p.tile([128, 512], F32, tag="pp")
                nc.tensor.matmul(pb[:, 0:96], W['Pd4'][i], W['xqb'], start=True,
                                 stop=False)
                nc.tensor.matmul(pb[:, 0:96], W['bpe'][i], ones96, start=False,
                                 stop=True)
                QP = blk.tile([128, 96], F32, tag="QP")
                nc.vector.tensor_copy(QP, pb[:, 0:96])
                return EK, VK, QB, QP

            def warm_pe(n, dep_row):
                # Back-to-back 384-col dummy matmuls keep the tensor engine's
                # p-state ramp alive through a tail bubble so the consts /
                # MLP / next-block matmuls run at full speed. dep_row gates
                # the burst start on a mid-tail tensor (bf16 [1,384] slice).
                for _ in range(n):
                    t = ps_p.tile([128, 512], F32, tag="pp")
                    nc.tensor.matmul(t[:, 0:384], wpack[0:1, 0:128], dep_row,
                                     start=True, stop=True)

            def emit_hpre_for(i, m, EK):
                s4o = 32 * (m % 3)
                t = ps_h.tile([128, 512], F32, tag="ph")
                nc.tensor.matmul(
                    t[:, 0:384],
                    wpack[s4o:s4o + 30, W_G1P + 128 * i: W_G1P + 128 * (i + 1)],
                    tsl_of(m), start=True, stop=False)
                nc.tensor.matmul(t[:, 0:384], W['Wg1'][i], EK,
                                 start=False, stop=True)
                return t

            # block-0 consts hoisted before the trig phase so the first PREQ
            # queries' hpre+relu pre-run in the trig shadow (PE/ACT have idle
            # slack there; Relu shares the Sin activation-table set).
            for ci in range(2):
                emit_trig_pair(ci)
            consts0 = make_consts(0)
            pre_h = {}
            for ci in range(2, NCH // 2):
                emit_trig_pair(ci)
                for m in (2 * (ci - 2), 2 * (ci - 2) + 1):
                    if m < PREQ:
                        hq = emit_hpre_for(0, m, consts0[0])
                        h_t = hp.tile([128, 384], BF16, tag="h")
                        nc.vector.tensor_scalar(
                            out=h_t, in0=hq[:, 0:384],
                            scalar1=consts0[2][:, m:m + 1], scalar2=0.0,
                            op0=OP.add, op1=OP.max)
                        pre_h[m] = h_t
            # Pin the natural_log_exp_and_others table (serves Exp/Relu/
            # Identity/Ln/Copy) once after the Sin phase so no further
            # LoadActFuncSet (1283 ns each) lands on block-tail critical paths.
            nc.scalar.add_instruction(mybir.InstLoadActFuncSet(
                name=nc.get_next_instruction_name(), act_func_set_id=6,
                ins=[], outs=[]))

            def affine_evict(src_ap, sc, b2, shape, dt=BF16, tag="aff", pool=None):
                t = (pool or fpool).tile(shape, dt, tag=tag)
                nc.scalar.activation(out=t, in_=src_ap, func=AF.Identity,
                                     bias=b2, scale=sc)
                return t

            magic = vpack[:, V_MAGIC: V_MAGIC + 1]

            def rsqrt_dve(var_ap, tag):
                # 1/sqrt(var+eps) fully on DVE (shift-seed + 1 Newton step);
                # avoids activations whose table loads (1283 ns each) would
                # land on the block-tail critical path.
                # var >> EPS=1e-5 for this model (unit-variance features),
                # so the +EPS add is dropped: error < 1e-5/var, far below
                # the 2e-2 budget. The Newton 0.5 factor folds into the
                # -0.5 constant of the polish step (one fewer DVE hop).
                y = smalls.tile([128, 1], F32, tag=tag + "y")
                nc.vector.tensor_scalar(out=y.bitcast(U32),
                                        in0=var_ap.bitcast(U32),
                                        scalar1=1, scalar2=None,
                                        op0=OP.logical_shift_right)
                nc.vector.tensor_tensor(out=y.bitcast(U32),
                                        in0=magic.bitcast(U32),
                                        in1=y.bitcast(U32), op=OP.subtract)
                a = smalls.tile([128, 1], F32, tag=tag + "a")
                nc.vector.tensor_tensor(out=a, in0=y, in1=y, op=OP.mult)
                nc.vector.tensor_tensor(out=a, in0=var_ap, in1=a, op=OP.mult)
                nc.vector.tensor_scalar(out=a, in0=a, scalar1=-0.5,
                                        scalar2=1.5, op0=OP.mult, op1=OP.add)
                nc.vector.tensor_tensor(out=y, in0=y, in1=a, op=OP.mult)
                return y

            # per-block fraction of relus on ACT (rest on DVE), tuned to
            # balance the engines: block 0's DVE also carries the trig
            # range-reduction, blocks 1-2 split ~78/22.
            def relu_on_act(i, m):
                if m % 2 == 0:
                    return True
                return ((m // 2) * 9) % 16 < 9

            # ---------- transformer blocks ----------
            for i in range(NB):
                if i == 0:
                    EK, VK, QB, QP = consts0
                else:
                    EK, VK, QB, QP = make_consts(i)

                S1 = blk.tile([128, 96], F32, tag="S1")
                S0 = blk.tile([128, 96], F32, tag="S0")
                payload = blk.tile([128, 96], BF16, tag="payload")
                o_full = wide.tile([128, 768], BF16, tag="ofull")
                sto = smalls.tile([128, 16, 6], F32, tag="sto")

                def gather_part(hs, hw, pt):
                    Rh = smalls.tile([128, hw], F32, tag=f"R{pt}")
                    nc.vector.reciprocal(out=Rh, in_=S0[:, hs:hs + hw])
                    nc.vector.tensor_tensor(out=payload[:, hs:hs + hw],
                                            in0=S1[:, hs:hs + hw], in1=Rh,
                                            op=OP.mult)
                    nc.vector.tensor_tensor(out=payload[:, hs:hs + hw],
                                            in0=payload[:, hs:hs + hw],
                                            in1=fq[:, hs:hs + hw], op=OP.add)
                    ag_in = dram.tile([128, hw], BF16, tag=f"agin{i}h{pt}")
                    if variant == "spmd":
                        ag_out = dram.tile([8, 128, hw], BF16,
                                           addr_space="Shared",
                                           tag=f"agout{i}h{pt}")
                    else:
                        ag_out = dram.tile([8, 128, hw], BF16,
                                           tag=f"agout{i}h{pt}")
                    nc.gpsimd.dma_start(out=ag_in, in_=payload[:, hs:hs + hw])
                    ago_ap = ag_out[:]
                    pay_ap = payload[:]
                    if variant == "spmd":
                        nc.gpsimd.collective_compute(
                            "AllGather", OP.bypass, replica_groups=RG,
                            ins=[ag_in[:].opt()], outs=[ag_out[:].opt()])
                    else:
                        bsrc = bass.AP(tensor=pay_ap.tensor,
                                       offset=pay_ap.offset + hs,
                                       ap=[list(pay_ap.ap[0]), [0, 8],
                                           [1, hw]])
                        bdst = bass.AP(tensor=ago_ap.tensor,
                                       offset=ago_ap.offset,
                                       ap=[[hw, 128], [128 * hw, 8], [1, hw]])
                        nc.sync.dma_start(out=bdst, in_=bsrc)
                    of_ap = o_full[:]
                    odst = bass.AP(tensor=of_ap.tensor,
                                   offset=of_ap.offset + hs,
                                   ap=[list(of_ap.ap[0]), [96, 8], [1, hw]])
                    src = bass.AP(tensor=ago_ap.tensor, offset=ago_ap.offset,
                                  ap=[[hw, 128], [128 * hw, 8], [1, hw]])
                    eng = nc.sync if pt == 0 else nc.scalar
                    eng.dma_start(out=odst, in_=src)
                    # stats over the freshly landed region (84- or 12-wide
                    # slices of each core's 96 columns, one stat group per
                    # core) so only the tiny second gather's stats sit on
                    # the block-tail critical path.
                    for g in range(8):
                        st_v = bass.AP(tensor=of_ap.tensor,
                                       offset=of_ap.offset + hs + 96 * g,
                                       ap=[list(of_ap.ap[0]), [1, hw]])
                        nc.vector.bn_stats(out=sto[:, 8 * pt + g, :], in_=st_v)

                # attention over 96 queries in 48 pairs.
                pre = dict(pre_h) if i == 0 else {}
                nxt = {}
                prime = (PREQ, PREQ + 1) if i == 0 else (0, 1)
                for m in prime:
                    if m not in pre:
                        nxt[m] = emit_hpre_for(i, m, EK)
                pend = []
                w_for = {}
                lg_prev = None

                def drain_one():
                    m = pend.pop(0)
                    w_sl = w_for.pop(m)
                    # S0 on DVE in 4x_2p mode: all-SBUF bf16 -> scratch,
                    # accumulator gives the per-query sum of w.
                    nc.vector.tensor_scalar(out=sdum, in0=w_sl,
                                            scalar1=1.0, scalar2=0.0,
                                            op0=OP.mult, op1=OP.add,
                                            accum_out=S0[:, m:m + 1])
                    pos = ps_p.tile([128, 512], F32, tag="pp")
                    s4o = 32 * (m % 3)
                    nc.tensor.matmul(
                        pos[:, 0:384],
                        wpack[s4o:s4o + 30, W_PS + 128 * i: W_PS + 128 * (i + 1)],
                        tsl_of(m), start=True, stop=False)
                    nc.tensor.matmul(pos[:, 0:384], W['i128'], VK,
                                     start=False, stop=True)
                    nc.vector.scalar_tensor_tensor(
                        out=dumA.broadcast_to((128, 384)),
                        in0=pos[:, 0:384], scalar=QP[:, m:m + 1],
                        in1=w_sl, op0=OP.add, op1=OP.mult,
                        accum_out=S1[:, m:m + 1])
                    if m == 83:
                        gather_part(0, 84, 0)

                def emit_relu(m):
                    if m in pre:
                        return pre.pop(m)
                    hpre = nxt.pop(m)
                    h_t = hp.tile([128, 384], BF16, tag="h")
                    if relu_on_act(i, m):
                        nc.scalar.activation(out=h_t, in_=hpre[:, 0:384],
                                             func=AF.Relu,
                                             bias=QB[:, m:m + 1], scale=1.0)
                    else:
                        nc.vector.tensor_scalar(
                            out=h_t, in0=hpre[:, 0:384],
                            scalar1=QB[:, m:m + 1], scalar2=0.0,
                            op0=OP.add, op1=OP.max)
                    return h_t
                for pr in range(NPAIR):
                    a, b = 2 * pr, 2 * pr + 1
                    h_a = emit_relu(a)
                    lg = ps_l.tile([128, 1024], F32, tag="lg")
                    nc.tensor.matmul(lg[:, 0:384], W['Wg2'][i], h_a,
                                     start=True, stop=True)
                    # exp for the PREVIOUS pair sits BETWEEN the two relus on
                    # ACT: its logits completed an iteration ago (no stall),
                    # it finishes early enough that the next iteration's lg
                    # matmuls never WAR-wait on it, and relu(b) stays early
                    # enough that lg(b) beats the exp that needs it next
                    # iteration. s0/stt drain one further pair behind.
                    if lg_prev is not None:
                        w_pair = wp.tile([128, 768], BF16, tag="w")
                        nc.scalar.activation(out=pair_view(w_pair[:], 384),
                                             in_=pair_view(lg_prev[:], 512),
                                             func=AF.Exp, bias=W['bg2'][i],
                                             scale=1.0)
                        w_for[a - 2] = w_pair[:, 0:384]
                        w_for[b - 2] = w_pair[:, 384:768]
                        pend.append(a - 2)
                        pend.append(b - 2)
                    lg_prev = lg
                    h_b = emit_relu(b)
                    nc.tensor.matmul(lg[:, 512:896], W['Wg2'][i], h_b,
                                     start=True, stop=True)
                    if pr == NPAIR - 1:
                        # final pair: collapse the software-pipeline lag so
                        # the flush is just the last two drains.
                        w_pair = wp.tile([128, 768], BF16, tag="w")
                        nc.scalar.activation(out=pair_view(w_pair[:], 384),
                                             in_=pair_view(lg[:], 512),
                                             func=AF.Exp, bias=W['bg2'][i],
                                             scale=1.0)
                        w_for[a] = w_pair[:, 0:384]
                        w_for[b] = w_pair[:, 384:768]
                        pend.append(a)
                        pend.append(b)
                        while pend:
                            drain_one()
                    else:
                        while len(pend) > 2:
                            drain_one()
                    for m in (a + 2, b + 2):
                        if m < NQ and m not in pre and m not in nxt:
                            nxt[m] = emit_hpre_for(i, m, EK)
                gather_part(84, 12, 1)
                warm_pe(40 if i == 0 else 12, w_pair[0:1, 0:384])

                # block tail via half-gathers: queries 0-83 are reduced,
                # all-gathered, written back and stats'd while 84-95 still
                # compute; only the 12-query gather's stats remain here.
                mv = smalls.tile([128, 2], F32, tag="bnmv")
                nc.vector.bn_aggr(out=mv, in_=sto)
                rs = rsqrt_dve(mv[:, 1:2], "bn")
                sc = smalls.tile([128, 1], F32, tag="sc")
                nc.vector.tensor_tensor(out=sc, in0=W['gam'][i], in1=rs, op=OP.mult)
                b2 = smalls.tile([128, 1], F32, tag="b2")
                nc.vector.tensor_scalar(out=b2, in0=mv[:, 0:1], scalar1=sc,
                                        scalar2=None, op0=OP.mult)
                nc.vector.tensor_tensor(out=b2, in0=W['bet'][i], in1=b2, op=OP.subtract)

                if i < NB - 1:
                    fq = fpool.tile([128, 96], BF16, tag="fq")
                    nc.vector.tensor_scalar(out=fq, in0=payload[:, 0:96],
                                            scalar1=sc, scalar2=b2,
                                            op0=OP.mult, op1=OP.add)
                if i == 0:
                    fb = fpool.tile([128, 384], BF16, tag="fb")
                    with tc.If(pid < 4) as cmp:
                        nc.scalar.activation(out=fb, in_=o_full[:, 0:384],
                                             func=AF.Identity, bias=b2, scale=sc)
                    with cmp.Else():
                        nc.scalar.activation(out=fb, in_=o_full[:, 384:768],
                                             func=AF.Identity, bias=b2, scale=sc)

                # ---------- MLP ----------
                if i > 0:
                    j = i - 1

                    def mlp_wide(lw, bias_ap, xin, tag):
                        # 768-wide layer: 2 matmul banks + ONE pair relu
                        t = wide.tile([128, 768], BF16, tag=tag)
                        pp = ps_l.tile([128, 1024], F32, tag="lg")
                        nc.tensor.matmul(pp[:, 0:384], lw, xin[:, 0:384],
                                         start=True, stop=True)
                        nc.tensor.matmul(pp[:, 512:896], lw, xin[:, 384:768],
                                         start=True, stop=True)
                        nc.scalar.activation(out=pair_view(t[:], 384),
                                             in_=pair_view(pp[:], 512),
                                             func=AF.Relu, bias=bias_ap,
                                             scale=1.0)
                        return t

                    def mlp_q(lw, bias_ap, xin, tag):
                        t = wide.tile([128, 96], BF16, tag=tag)
                        pp = ps_h.tile([128, 512], F32, tag="ph")
                        nc.tensor.matmul(pp[:, 0:96], lw, xin, start=True,
                                         stop=True)
                        nc.scalar.activation(out=t, in_=pp[:, 0:96],
                                             func=AF.Relu, bias=bias_ap,
                                             scale=1.0)
                        return t

                    # fold the BN scale into em1 (per input channel =
                    # per partition of the transposed weights) so the MLP
                    # consumes the gathered bf16 o_full directly -- the
                    # f_full eviction leaves the tail critical path. The
                    # per-channel BN shift b2 is folded into the layer-1
                    # bias; it is NOT added to the residual (o2 = sc*o+y2),
                    # which cancels exactly through the next BatchNorm's own
                    # mean subtraction.
                    em1s = smalls.tile([128, 128], BF16, tag="em1s")
                    nc.vector.tensor_scalar(out=em1s, in0=W['em1'][j],
                                            scalar1=sc, scalar2=None,
                                            op0=OP.mult)
                    b2bf = smalls.tile([128, 1], BF16, tag="b2bf")
                    nc.vector.tensor_scalar(out=b2bf, in0=b2, scalar1=1.0,
                                            scalar2=None, op0=OP.mult)
                    pbias = ps_h.tile([128, 512], F32, tag="ph")
                    nc.tensor.matmul(pbias[:, 0:1], W['em1'][j], b2bf,
                                     start=True, stop=True)
                    bias1 = smalls.tile([128, 1], F32, tag="bias1")
                    nc.vector.tensor_tensor(out=bias1, in0=pbias[:, 0:1],
                                            in1=W['emb1'][j], op=OP.add)
                    y1f = mlp_wide(em1s, bias1, o_full, "y1f")
                    y2f = mlp_wide(W['em2'][j], W['emb2'][j], y1f, "y2f")
                    o2f = wide.tile([128, 768], BF16, tag="o2f")
                    o_s = wide.tile([128, 768], BF16, tag="o_s")
                    for hh in range(2):
                        sl = slice(384 * hh, 384 * hh + 384)
                        nc.vector.tensor_scalar(out=o_s[:, sl],
                                                in0=o_full[:, sl], scalar1=sc,
                                                scalar2=None, op0=OP.mult)
                        nc.vector.tensor_tensor(out=o2f[:, sl], in0=o_s[:, sl],
                                                in1=y2f[:, sl], op=OP.add)
                    warm_pe(22, o2f[0:1, 0:384])
                    if i < NB - 1:
                        y1q = mlp_q(W['em1'][j], W['emb1'][j], fq, "y1q")
                        y2q = mlp_q(W['em2'][j], W['emb2'][j], y1q, "y2q")
                        o2q = wide.tile([128, 96], BF16, tag="o2q")
                        nc.vector.scalar_tensor_tensor(
                            out=o2q, in0=payload[:, 0:96], scalar=sc,
                            in1=y2q, op0=OP.mult, op1=OP.add)

                    st2 = smalls.tile([128, 2, 6], F32, tag="st2")
                    nc.vector.bn_stats(out=st2[:, 0, :], in_=o2f[:, 0:384])
                    nc.vector.bn_stats(out=st2[:, 1, :], in_=o2f[:, 384:768])
                    mv2 = smalls.tile([128, 2], F32, tag="mv2")
                    nc.vector.bn_aggr(out=mv2, in_=st2)
                    rs2 = rsqrt_dve(mv2[:, 1:2], "em")
                    sc2 = smalls.tile([128, 1], F32, tag="sc")
                    nc.vector.tensor_tensor(out=sc2, in0=W['emg'][j], in1=rs2,
                                            op=OP.mult)
                    b22 = smalls.tile([128, 1], F32, tag="b2")
                    nc.vector.tensor_scalar(out=b22, in0=mv2[:, 0:1], scalar1=sc2,
                                            scalar2=None, op0=OP.mult)
                    nc.vector.tensor_tensor(out=b22, in0=W['embe'][j], in1=b22,
                                            op=OP.subtract)
                    if i == NB - 1:
                        f_full = fpool.tile([128, 768], BF16, tag="ffull")
                        nc.scalar.activation(out=f_full[:, 0:384],
                                             in_=o2f[:, 0:384],
                                             func=AF.Identity, bias=b22,
                                             scale=sc2)
                        nc.scalar.activation(out=f_full[:, 384:768],
                                             in_=o2f[:, 384:768],
                                             func=AF.Identity, bias=b22,
                                             scale=sc2)
                    if i < NB - 1:
                        fb = fpool.tile([128, 384], BF16, tag="fb")
                        with tc.If(pid < 4) as cmp:
                            nc.scalar.activation(out=fb, in_=o2f[:, 0:384],
                                                 func=AF.Identity, bias=b22,
                                                 scale=sc2)
                        with cmp.Else():
                            nc.scalar.activation(out=fb, in_=o2f[:, 384:768],
                                                 func=AF.Identity, bias=b22,
                                                 scale=sc2)
                        fq = fpool.tile([128, 96], BF16, tag="fq")
                        nc.vector.tensor_scalar(out=fq, in0=o2q, scalar1=sc2,
                                                scalar2=b22, op0=OP.mult,
                                                op1=OP.add)

            # ---------- final FC + max ----------
            ot4 = sing.tile([128, 4], F32, tag="ot4")
            for bb in range(2):
                fbb = f_full[:, bb * 384:(bb + 1) * 384]
                pp = ps_l.tile([128, 1024], F32, tag="lg")
                nc.tensor.matmul(pp[:, 0:384], W['f1'][0], fbb, start=True,
                                 stop=True)
                nc.tensor.matmul(pp[:, 512:896], W['f1'][1], fbb, start=True,
                                 stop=True)
                e1 = wide.tile([128, 768], BF16, tag="e1")
                e1_v = bass.AP(tensor=e1[:].tensor, offset=e1[:].offset,
                               ap=[list(e1[:].ap[0]), [384, 2], [1, 384]])
                # per-half bias differs (f1b[0]/f1b[1]); pair relu needs one
                # bias, so split the relu in two over the wide psum tile.
                nc.scalar.activation(out=e1[:, 0:384], in_=pp[:, 0:384],
                                     func=AF.Relu, bias=W['f1b'][0], scale=1.0)
                nc.vector.tensor_scalar(out=e1[:, 384:768], in0=pp[:, 512:896],
                                        scalar1=W['f1b'][1], scalar2=0.0,
                                        op0=OP.add, op1=OP.max)
                pq = ps_l.tile([128, 1024], F32, tag="lg")
                for h in range(2):
                    nc.tensor.matmul(pq[:, 512 * h: 512 * h + 384],
                                     W['f2'][h][0], e1[:, 0:384],
                                     start=True, stop=False)
                    nc.tensor.matmul(pq[:, 512 * h: 512 * h + 384],
                                     W['f2'][h][1], e1[:, 384:768],
                                     start=False, stop=True)
                mx = smalls.tile([128, 2], F32, tag="mx")
                nc.vector.tensor_reduce(out=mx, in_=pair_view(pq[:], 512),
                                        axis=mybir.AxisListType.X, op=OP.max)
                for h in range(2):
                    nc.vector.tensor_scalar(out=ot4[:, 2 * bb + h: 2 * bb + h + 1],
                                            in0=mx[:, h:h + 1], scalar1=W['f2b'][h],
                                            scalar2=None, op0=OP.add)
                od_ap = out_d[:]
                odst = bass.AP(tensor=od_ap.tensor,
                               offset=od_ap.offset + 256 * bb,
                               ap=[[1, 128], [128, 2]])
                nc.sync.dma_start(out=odst, in_=ot4[:, 2 * bb: 2 * bb + 2])

    nc.compile()
    _CACHE[variant] = nc
    return nc


def _prep_inputs(inputs):
    """Host-side constant relayout + per-core packing. Returns in_maps list."""
    xyz = _f32(inputs["xyz"])          # [2, 384, 3]
    feats = _f32(inputs["feats"])      # [2, 384, 1]

    Wq, Wk, Wv = inputs["tb_Wq"], inputs["tb_Wk"], inputs["tb_Wv"]
    Wg1, bg1 = inputs["tb_Wg1"], inputs["tb_bg1"]
    Wg2, bg2 = inputs["tb_Wg2"], inputs["tb_bg2"]
    Wpe, bpe = inputs["tb_Wpe"], inputs["tb_bpe"]

    wpack = np.zeros((128, WCOLS), np.float32)
    vpack = np.zeros((128, VCOLS), np.float32)
    rpack_c = np.zeros((3, RCOLS), np.float32)   # per-core cols filled later
    qpack_c = np.zeros((1, QCOLS), np.float32)

    for i in range(NB):
        Ws, Wd = _wpe_split(_f32(Wpe[i]))
        g1 = _f32(Wg1[i])
        wpack[:, W_WG1 + 128 * i: W_WG1 + 128 * (i + 1)] = g1.T
        wpack[:, W_WG2 + 128 * i: W_WG2 + 128 * (i + 1)] = _f32(Wg2[i]).T
        wpack[:, W_NWK + 128 * i: W_NWK + 128 * (i + 1)] = (-_f32(Wk[i])).T
        wpack[:, W_WV + 128 * i: W_WV + 128 * (i + 1)] = _f32(Wv[i]).T
        wpack[:, W_G1Q + 128 * i: W_G1Q + 128 * (i + 1)] = (g1 @ _f32(Wq[i])).T
        wpack[:, W_G1P + 128 * i: W_G1P + 128 * (i + 1)] = _rep3((g1 @ Ws).T)
        wpack[:, W_PS + 128 * i: W_PS + 128 * (i + 1)] = _rep3(Ws.T)
        rpack_c[:, R_NPD4 + 128 * i: R_NPD4 + 128 * (i + 1)] = (-4.0 * Wd).T
        rpack_c[:, R_G1PD4 + 128 * i: R_G1PD4 + 128 * (i + 1)] = (4.0 * (g1 @ Wd)).T
        rpack_c[:, R_PD4 + 128 * i: R_PD4 + 128 * (i + 1)] = (4.0 * Wd).T
        qpack_c[0, Q_C1 + 128 * i: Q_C1 + 128 * (i + 1)] = g1 @ _f32(bpe[i]) + _f32(bg1[i])
        qpack_c[0, Q_BPE + 128 * i: Q_BPE + 128 * (i + 1)] = _f32(bpe[i])
        vpack[:, V_BG2 + i] = _f32(bg2[i])
        vpack[:, V_GAM + i] = _f32(inputs["tb_gamma"][i])
        vpack[:, V_BET + i] = _f32(inputs["tb_beta"][i])

    wpack[:, W_I128: W_I128 + 128] = np.eye(128, dtype=np.float32)
    for j in range(NF):
        wpack[:, W_EM1 + 128 * j: W_EM1 + 128 * (j + 1)] = _f32(inputs["em_W1"][j]).T
        wpack[:, W_EM2 + 128 * j: W_EM2 + 128 * (j + 1)] = _f32(inputs["em_W2"][j]).T
        vpack[:, V_EMB1 + j] = _f32(inputs["em_b1"][j])
        vpack[:, V_EMB2 + j] = _f32(inputs["em_b2"][j])
        vpack[:, V_EMG + j] = _f32(inputs["em_gamma"][j])
        vpack[:, V_EMBE + j] = _f32(inputs["em_beta"][j])
    W1T = _f32(inputs["fcf_W1"]).T           # [128, 256]
    for h in range(2):
        wpack[:, W_F1 + 128 * h: W_F1 + 128 * (h + 1)] = W1T[:, h * 128:(h + 1) * 128]
        vpack[:, V_F1B + h] = _f32(inputs["fcf_b1"])[h * 128:(h + 1) * 128]
        vpack[:, V_F2B + h] = _f32(inputs["fcf_b2"])[h * 128:(h + 1) * 128]
    W2T = _f32(inputs["fcf_W2"]).T           # [256, 256]
    for h in range(2):
        for k in range(2):
            wpack[:, W_F2 + 128 * (2 * h + k): W_F2 + 128 * (2 * h + k + 1)] = \
                W2T[k * 128:(k + 1) * 128, h * 128:(h + 1) * 128]
    vpack[:, V_ENCB] = _f32(inputs["enc_b"])
    vpack[:, V_MAGIC] = np.array([0x5F3759DF], np.uint32).view(np.float32)[0]
    qpack_c[0, Q_FEATS: Q_FEATS + 768] = feats.reshape(768)
    qpack_c[0, Q_ENC: Q_ENC + 128] = _f32(inputs["enc_W"])[:, 0]

    wpack_b = _bf(wpack)
    vpack_f = _f32(vpack)

    # s coefficients: r = s*xk - (s*xq - off), s = 4*freq/2pi
    svals = (4.0 * FREQS / TWO_PI)  # [5] f64

    in_maps = []
    for c in range(8):
        b, qo = c // 4, (c % 4) * 96
        xk = xyz[b].T                      # [3, 384]

        # fpack: bf16 split-precision trig matmul operands.
        # rhs rows: 0-14 z_hi (5j+f), 15-29 z_lo, 30-31 ones.
        xko = np.zeros((32, 384), np.float32)
        for j in range(3):
            for f in range(5):
                zv = svals[f] * xyz[b, :, j].astype(np.float64)  # [384]
                zh = np.float32(np.asarray(zv, np.float32).astype(BF))
                zl = (zv - zh).astype(np.float32).astype(BF)
                xko[5 * j + f] = zh
                xko[15 + 5 * j + f] = np.float32(zl)
        xko[30] = 1.0
        xko[31] = 1.0

        # lhsT: selector rows pair z_hi and z_lo; c rows carry the
        # per-query constant split hi/lo.
        S4 = np.zeros((32, NCH * 96), np.float32)
        for cch in range(NCH):
            for s in range(3):
                qg = qo + 3 * cch + s
                for j in range(3):
                    for t in range(10):
                        col = 96 * cch + 32 * s + 10 * j + t
                        f = t % 5
                        S4[5 * j + f, col] = 1.0
                        S4[15 + 5 * j + f, col] = 1.0
                        cval = (0.25 if t >= 5 else 0.0) - \
                            svals[f] * np.float64(xyz[b, qg, j])
                        chv = np.float32(np.asarray(cval, np.float32).astype(BF))
                        clv = np.float32(np.float32(cval - chv).astype(BF))
                        S4[30, col] = chv
                        S4[31, col] = clv
        fpack = np.zeros((32, FCOLS), np.float32)
        fpack[:, F_S4: F_S4 + NCH * 96] = S4
        fpack[:, F_XKO: F_XKO + 384] = xko

        rpack = rpack_c.copy()
        rpack[:, R_XKB: R_XKB + 384] = xk
        rpack[:, R_XQB: R_XQB + 96] = xk[:, qo:qo + 96]
        qpack = qpack_c.copy()
        qpack[0, Q_FB: Q_FB + 384] = feats[b].reshape(384)
        qpack[0, Q_FQ: Q_FQ + 96] = feats[b, qo:qo + 96].reshape(96)

        in_maps.append({
            "wpack": wpack_b,
            "vpack": vpack_f,
            "rpack": _bf(rpack),
            "qpack": _bf(qpack),
            "fpack": _bf(fpack),
        })
    return in_maps


def kernel(**inputs):
    from concourse.bass_utils import run_bass_kernel_spmd

    nc = _build()
    in_maps = _prep_inputs(inputs)
    res = run_bass_kernel_spmd(nc, in_maps, list(range(8)))
    return np.asarray(res.results[0]["out"], np.float32)


if __name__ == "__main__":
    rng = np.random.RandomState(0)
    fake = {
        "xyz": rng.randn(2, 384, 3).astype(np.float32),
        "feats": rng.randn(2, 384, 1).astype(np.float32),
    }
    print("smoke build only")
